# revision 36
# baseline (speedup 1.0000x reference)
"""Multi-head linear self-attention (ELU+1 feature map) — Trainium2 Bass kernel.

Reference computation (b=4, n=4096, f=768, h=12, d=64):
    q = phi(x@Wq + bq), k = phi(x@Wk + bk), v = x@Wv + bv   with phi = elu+1
    kv[h] = k[h].T @ v[h]  (sum over full sequence)
    ksum[h] = sum_n k[h]
    z = 1/(q . ksum);  out = concat_h(q[h] @ kv[h] * z) @ Wo + bo

Sharding: 8 cores = batch(4) x head-half(2). Each core gets one batch element
and a 6-head column-slice of Wq/Wk/Wv (+ the matching row-slice of Wo) and
produces a partial output [4096, 768]. Host unshard = sum of the two partials
per batch (row-parallel tensor parallelism). bo is folded in by feeding the
real bo to even cores and zeros to odd cores, keeping the program pure SPMD.

Per-core dataflow (all matmuls via PE with float32r fast mode):
    xT   = x.T via PE transposes                        [768, 4096] (feat-major)
    K,V  = row-major projections (lhsT = xT blocks)     [rows, 384]
    kv   accumulated in PSUM over 32 row chunks; a ones-column appended to V
         gives ksum for free.
    Q.T  = feat-major projection (lhsT = Wq, rhs = xT), phi fused into eviction
    z    = 1/(ksum_blockdiag.T @ Q.T), num.T = kvblockdiag.T @ Q.T
    out  = (num.T * z).T @ Wo via lhsT = normed.T blocks -> row-major PSUM -> DMA

phi(t) = elu(t)+1 = max(min(exp(t), 1), t+1).
"""

import os
from contextlib import ExitStack

import numpy as np

import concourse.bass as bass
import concourse.mybir as mybir
import concourse.tile as tile
from concourse import bacc
from concourse.bass_utils import run_bass_kernel_spmd
from concourse.masks import make_identity

FP = mybir.dt.float32
FPR = mybir.dt.float32r
ALU = mybir.AluOpType
ACTF = mybir.ActivationFunctionType

P = 128
R = 4096          # sequence rows per core (one full batch element)
F = 768           # input features
H = 6             # heads per core
D = 64            # head dim
G = H * D         # 384 output features per core
KO = F // P       # 6 input-feature chunks
MO = G // P       # 3 output-feature chunks
NCH = R // P      # 32 row chunks of 128
RC = 512          # stage-C row chunk
NRC = R // RC     # 8

N_CORES = 8


def _mm(ap):
    """Matmul operands are already float32r-typed tiles; identity hook kept so
    the dtype strategy can be swapped in one place."""
    return ap


def build_nc():
    nc = bacc.Bacc("TRN2", target_bir_lowering=False, debug=False)

    # Matmul-feeding tensors are float32r (same 4-byte layout as fp32): the PE
    # single-pass fp32 mode. Compute producers round on write; DRAM inputs are
    # fp32 bit patterns reinterpreted.
    x = nc.dram_tensor("x", [R, F], FP, kind="ExternalInput").ap()
    wq = nc.dram_tensor("wq", [F, G], FPR, kind="ExternalInput").ap()
    wk = nc.dram_tensor("wk", [F, G], FPR, kind="ExternalInput").ap()
    wv = nc.dram_tensor("wv", [F, G], FPR, kind="ExternalInput").ap()
    wo = nc.dram_tensor("wo", [G, F], FPR, kind="ExternalInput").ap()
    bq = nc.dram_tensor("bq", [G], FP, kind="ExternalInput").ap()
    bk = nc.dram_tensor("bk", [G], FP, kind="ExternalInput").ap()
    bv = nc.dram_tensor("bv", [G], FP, kind="ExternalInput").ap()
    bo = nc.dram_tensor("bo", [F], FP, kind="ExternalInput").ap()
    # esel[h, p, m] = 1 if h == 2p + (m >= 64): expands z [H, rc] to [128, rc]
    # per head-pair via a tiny matmul (partition-dim broadcast isn't allowed on
    # compute engines).
    esel = nc.dram_tensor("esel", [H, MO, P], FPR, kind="ExternalInput").ap()
    y = nc.dram_tensor("y", [R, F], FP, kind="ExternalOutput").ap()

    def bcast_rows(vec_ap, n):
        # DRAM [n] -> partition-broadcast AP [P, n] (partition stride 0)
        return bass.AP(tensor=vec_ap.tensor, offset=vec_ap.offset, ap=[[0, P], [1, n]])

    with TileCtx(nc) as tc, ExitStack() as ctx:
        singles = ctx.enter_context(tc.tile_pool(name="singles", bufs=1))
        wpool = ctx.enter_context(tc.tile_pool(name="wpool", bufs=2))

        ident = singles.tile([P, P], FP)
        make_identity(nc, ident)

        # biases
        bk_rep = singles.tile([P, G], FP, tag="bk_rep")
        nc.sync.dma_start(bk_rep, bcast_rows(bk, G))
        bv_rep = singles.tile([P, G], FP, tag="bv_rep")
        nc.sync.dma_start(bv_rep, bcast_rows(bv, G))
        bo_rep = singles.tile([P, F], FP, tag="bo_rep")
        nc.sync.dma_start(bo_rep, bcast_rows(bo, F))
        bq_col = singles.tile([P, MO], FP, tag="bq_col")
        nc.sync.dma_start(bq_col, bq.rearrange("(mo p) -> p mo", p=P))
        bq1_col = singles.tile([P, MO], FP, tag="bq1_col")
        nc.vector.tensor_scalar(bq1_col, bq_col, 1.0, None, op0=ALU.add)
        esel_sb = singles.tile([H, MO, P], FPR, tag="esel_sb")
        nc.sync.dma_start(esel_sb, esel)
        # fp32 constant tiles; memset on float32r tiles fails the ISA check,
        # so f32r destinations are zeroed/oned via rounding tensor_copy
        zeros_fp = singles.tile([P, P], FP, tag="zeros_fp")
        nc.vector.memset(zeros_fp, 0.0)
        ones_col = singles.tile([P, H, 1], FP, tag="ones_col")
        nc.vector.memset(ones_col, 1.0)

        # x.T, resident through stage C
        xt_pool = ctx.enter_context(tc.tile_pool(name="xt", bufs=1))
        xt = xt_pool.tile([P, KO, R], FPR)

        # ---------------- stage A0: build x.T via PE transposes -------------
        with ExitStack() as sctx:
            xst_pool = sctx.enter_context(tc.tile_pool(name="xst", bufs=3))
            tp_pool = sctx.enter_context(tc.tile_pool(name="tp", bufs=4, space="PSUM"))
            for i in range(NCH):
                xstage = xst_pool.tile([P, F], FP)
                nc.sync.dma_start(xstage, x[i * P : (i + 1) * P, :])
                for ko in range(KO):
                    tp = tp_pool.tile([P, P], FP)
                    nc.tensor.transpose(tp, xstage[:, ko * P : (ko + 1) * P], ident)
                    nc.vector.tensor_copy(xt[:, ko, i * P : (i + 1) * P], tp)

        # weights for stage A1
        wk_sb = wpool.tile([P, KO, G], FPR, tag="w")
        nc.sync.dma_start(wk_sb, wk.rearrange("(ko p) g -> p ko g", p=P))
        wv_sb = wpool.tile([P, KO, G], FPR, tag="w")
        nc.sync.dma_start(wv_sb, wv.rearrange("(ko p) g -> p ko g", p=P))

        # stage-A outputs that persist into stage C
        kvblk = [
            singles.tile([P, P], FPR, tag=f"kvblk{p}", name=f"kvblk{p}")
            for p in range(MO)
        ]
        ksum_mat = singles.tile([P, MO, H], FPR, tag="ksum_mat")

        # ---------------- stage A1: K, V, kv, ksum --------------------------
        with ExitStack() as sctx:
            kp_pool = sctx.enter_context(tc.tile_pool(name="kp", bufs=2, space="PSUM"))
            vp_pool = sctx.enter_context(tc.tile_pool(name="vp", bufs=2, space="PSUM"))
            kv_pool = sctx.enter_context(tc.tile_pool(name="kvp", bufs=1, space="PSUM"))
            ksb_pool = sctx.enter_context(tc.tile_pool(name="ksb", bufs=2))
            vsb_pool = sctx.enter_context(tc.tile_pool(name="vsb", bufs=2))
            tmp_pool = sctx.enter_context(tc.tile_pool(name="katmp", bufs=2))

            kv_ps = [
                kv_pool.tile([P, H * (D + 1)], FP, tag=f"kv{p}", name=f"kv{p}")
                for p in range(MO)
            ]

            for i in range(NCH):
                # K projection: rows on partitions
                kps = kp_pool.tile([P, G], FP)
                for ko in range(KO):
                    nc.tensor.matmul(
                        kps,
                        lhsT=_mm(xt[:, ko, i * P : (i + 1) * P]),
                        rhs=_mm(wk_sb[:, ko, :]),
                        start=(ko == 0),
                        stop=(ko == KO - 1),
                    )
                # phi(k + bk) = max(min(exp(t), 1), t + 1)
                t1 = tmp_pool.tile([P, G], FP, tag="kt1")
                nc.vector.tensor_tensor(t1, kps, bk_rep, op=ALU.add)
                e = tmp_pool.tile([P, G], FP, tag="ke")
                nc.scalar.activation(e, t1, ACTF.Exp)
                nc.gpsimd.tensor_scalar_min(e, e, 1.0)
                ksb = ksb_pool.tile([P, G], FPR)
                nc.vector.tensor_scalar(ksb, t1, 1.0, None, op0=ALU.add)
                nc.vector.tensor_tensor(ksb, ksb, e, op=ALU.max)

                # V projection + bias, with ones column per head for ksum
                vps = vp_pool.tile([P, G], FP)
                for ko in range(KO):
                    nc.tensor.matmul(
                        vps,
                        lhsT=_mm(xt[:, ko, i * P : (i + 1) * P]),
                        rhs=_mm(wv_sb[:, ko, :]),
                        start=(ko == 0),
                        stop=(ko == KO - 1),
                    )
                vext = vsb_pool.tile([P, H, D + 1], FPR)
                nc.vector.tensor_copy(vext[:, :, D : D + 1], ones_col)
                nc.vector.tensor_tensor(
                    vext[:, :, 0:D],
                    vps.rearrange("p (h d) -> p h d", d=D),
                    bv_rep.rearrange("p (h d) -> p h d", d=D),
                    op=ALU.add,
                )

                # kv accumulation: one matmul per head-pair, full V as rhs
                vflat = vext.rearrange("p h e -> p (h e)")
                for p in range(MO):
                    nc.tensor.matmul(
                        kv_ps[p],
                        lhsT=_mm(ksb[:, p * P : (p + 1) * P]),
                        rhs=_mm(vflat),
                        start=(i == 0),
                        stop=(i == NCH - 1),
                    )

            # extract kv block-diagonals and ksum columns
            for p in range(MO):
                nc.vector.tensor_copy(kvblk[p], zeros_fp)
                h0, h1 = 2 * p, 2 * p + 1
                nc.vector.tensor_copy(
                    kvblk[p][0:D, 0:D], kv_ps[p][0:D, h0 * (D + 1) : h0 * (D + 1) + D]
                )
                nc.vector.tensor_copy(
                    kvblk[p][D:P, D:P], kv_ps[p][D:P, h1 * (D + 1) : h1 * (D + 1) + D]
                )
            nc.vector.tensor_copy(
                ksum_mat, zeros_fp[:, 0 : MO * H].rearrange("p (mo h) -> p mo h", h=H)
            )
            for h in range(H):
                p = h // 2
                r0 = (h % 2) * D
                nc.vector.tensor_copy(
                    ksum_mat[r0 : r0 + D, p, h : h + 1], kv_ps[p][r0 : r0 + D, D : D + 1]
                )

        # weights for stage C
        wq_sb = wpool.tile([P, KO, G], FPR, tag="w")
        nc.sync.dma_start(wq_sb, wq.rearrange("(ko p) g -> p ko g", p=P))
        wo_sb = wpool.tile([P, MO, F], FPR, tag="w")
        nc.sync.dma_start(wo_sb, wo.rearrange("(mo p) f -> p mo f", p=P))

        # ---------------- stage C: Q.T, z, num, output ----------------------
        with ExitStack() as sctx:
            qp_pool = sctx.enter_context(tc.tile_pool(name="qp", bufs=2, space="PSUM"))
            zp_pool = sctx.enter_context(tc.tile_pool(name="zp", bufs=1, space="PSUM"))
            np_pool = sctx.enter_context(tc.tile_pool(name="nump", bufs=2, space="PSUM"))
            zx_pool = sctx.enter_context(tc.tile_pool(name="zx", bufs=1, space="PSUM"))
            op_pool = sctx.enter_context(tc.tile_pool(name="outp", bufs=1, space="PSUM"))
            qt_pool = sctx.enter_context(tc.tile_pool(name="qt", bufs=2))
            qe_pool = sctx.enter_context(tc.tile_pool(name="qe", bufs=3))
            zr_pool = sctx.enter_context(tc.tile_pool(name="zr", bufs=2))
            zxs_pool = sctx.enter_context(tc.tile_pool(name="zxs", bufs=2))
            nrm_pool = sctx.enter_context(tc.tile_pool(name="nrm", bufs=2))
            out_pool = sctx.enter_context(tc.tile_pool(name="osb", bufs=3))

            for rc in range(NRC):
                rs = slice(rc * RC, (rc + 1) * RC)
                # Q.T chunk [128, MO, RC], phi fused into eviction
                qt_rc = qt_pool.tile([P, MO, RC], FPR)
                for mo in range(MO):
                    qps = qp_pool.tile([P, RC], FP)
                    for ko in range(KO):
                        nc.tensor.matmul(
                            qps,
                            lhsT=_mm(wq_sb[:, ko, mo * P : (mo + 1) * P]),
                            rhs=_mm(xt[:, ko, rs]),
                            start=(ko == 0),
                            stop=(ko == KO - 1),
                        )
                    e = qe_pool.tile([P, RC], FP)
                    nc.scalar.activation(e, qps, ACTF.Exp, bias=bq_col[:, mo : mo + 1])
                    nc.gpsimd.tensor_scalar_min(e, e, 1.0)
                    qslice = qt_rc[:, mo, :]
                    nc.vector.tensor_scalar(
                        qslice, qps, bq1_col[:, mo : mo + 1], None, op0=ALU.add
                    )
                    nc.vector.tensor_tensor(qslice, qslice, e, op=ALU.max)

                # z = 1 / (q . ksum)
                zps = zp_pool.tile([H, RC], FP)
                for mo in range(MO):
                    nc.tensor.matmul(
                        zps,
                        lhsT=_mm(ksum_mat[:, mo, :]),
                        rhs=_mm(qt_rc[:, mo, :]),
                        start=(mo == 0),
                        stop=(mo == MO - 1),
                    )
                zr = zr_pool.tile([H, RC], FPR)
                with nc.allow_low_precision(reason="fp32r rounding of 1/z"):
                    nc.vector.reciprocal(zr, zps)

                # num.T per head pair; z expanded across partitions via esel
                nrm = nrm_pool.tile([P, MO, RC], FPR)
                for p in range(MO):
                    nps = np_pool.tile([P, RC], FP)
                    nc.tensor.matmul(nps, lhsT=_mm(kvblk[p]), rhs=_mm(qt_rc[:, p, :]))
                    zxp = zx_pool.tile([P, RC], FP)
                    nc.tensor.matmul(zxp, lhsT=_mm(esel_sb[:, p, :]), rhs=_mm(zr))
                    zxs = zxs_pool.tile([P, RC], FPR)
                    nc.scalar.copy(zxs, zxp)
                    nc.vector.tensor_tensor(nrm[:, p, :], nps, zxs, op=ALU.mult)

                # output projection, row-major
                for sub in range(4):
                    o_ps = [
                        op_pool.tile([P, F // 2], FP, tag=f"op{hh}", name=f"ops{hh}")
                        for hh in range(2)
                    ]
                    for hh in range(2):
                        for p in range(MO):
                            nc.tensor.matmul(
                                o_ps[hh],
                                lhsT=_mm(nrm[:, p, sub * P : (sub + 1) * P]),
                                rhs=_mm(wo_sb[:, p, hh * (F // 2) : (hh + 1) * (F // 2)]),
                                start=(p == 0),
                                stop=(p == MO - 1),
                            )
                    osb = out_pool.tile([P, F], FP)
                    nc.vector.tensor_tensor(
                        osb[:, 0 : F // 2], o_ps[0], bo_rep[:, 0 : F // 2], op=ALU.add
                    )
                    nc.vector.tensor_tensor(
                        osb[:, F // 2 : F], o_ps[1], bo_rep[:, F // 2 : F], op=ALU.add
                    )
                    r0 = rc * RC + sub * P
                    nc.sync.dma_start(y[r0 : r0 + P, :], osb)

    nc.compile()
    return nc


def TileCtx(nc):
    return tile.TileContext(nc)


def make_in_maps(x, Wq, bq, Wk, bk, Wv, bv, Wo, bo):
    """Shard full inputs into the 8 per-core input maps."""
    f32 = lambda a: np.ascontiguousarray(np.asarray(a, dtype=np.float32))
    x, Wq, bq, Wk, bk, Wv, bv, Wo, bo = map(f32, (x, Wq, bq, Wk, bk, Wv, bv, Wo, bo))
    zeros_f = np.zeros_like(bo)
    esel = np.zeros((H, MO, P), dtype=np.float32)
    for h in range(H):
        esel[h, h // 2, (h % 2) * D : (h % 2 + 1) * D] = 1.0
    in_maps = []
    for c in range(N_CORES):
        b, g = divmod(c, 2)
        sl = slice(g * G, (g + 1) * G)
        in_maps.append(
            {
                "x": x[b],
                "wq": f32(Wq[:, sl]),
                "wk": f32(Wk[:, sl]),
                "wv": f32(Wv[:, sl]),
                "wo": f32(Wo[sl, :]),
                "bq": f32(bq[sl]),
                "bk": f32(bk[sl]),
                "bv": f32(bv[sl]),
                "bo": bo if g == 0 else zeros_f,
                "esel": esel,
            }
        )
    return in_maps


def unshard(core_outs):
    """Sum the two row-parallel partials per batch element."""
    return np.stack(
        [core_outs[2 * b] + core_outs[2 * b + 1] for b in range(N_CORES // 2)]
    )


_NC_CACHE = {}


def get_nc():
    if "nc" not in _NC_CACHE:
        _NC_CACHE["nc"] = build_nc()
    return _NC_CACHE["nc"]


def run(inputs, trace=False, **kwargs):
    nc = get_nc()
    in_maps = make_in_maps(**inputs)
    res = run_bass_kernel_spmd(
        nc, in_maps, core_ids=list(range(N_CORES)), trace=trace, **kwargs
    )
    out = unshard([r["y"] for r in res.results])
    return out, res


def kernel(**inputs):
    out, _ = run(inputs, trace=False)
    return out


# revision 37
# speedup vs baseline: 2.0323x; 2.0323x over previous
"""Multi-head linear self-attention (ELU+1 feature map) — Trainium2 Bass kernel.

Reference computation (b=4, n=4096, f=768, h=12, d=64):
    q = phi(x@Wq + bq), k = phi(x@Wk + bk), v = x@Wv + bv   with phi = elu+1
    kv[h] = k[h].T @ v[h]  (sum over full sequence)
    ksum[h] = sum_n k[h]
    z = 1/(q . ksum);  out = concat_h(q[h] @ kv[h] * z) @ Wo + bo

Sharding: 8 cores = batch(4) x head-half(2). Each core gets one batch element
and a 6-head column-slice of Wq/Wk/Wv (+ the matching row-slice of Wo) and
produces a partial output [4096, 768]. Host unshard = sum of the two partials
per batch (row-parallel tensor parallelism). bo is folded in by feeding the
real bo to even cores and zeros to odd cores, keeping the program pure SPMD.

Matmul operands are bf16 (host-side cast; PSUM accumulation stays fp32):
bf16 runs the PE at 1 cyc/row with fast weight load, while fp32/f32r stalls
~330 ns per LDWEIGHTS, and bf16 unlocks the DMA xbar transpose so x.T costs
zero PE/DVE work. Biases and the output path stay fp32.

Per-core dataflow:
    xT   = x.T via 6 DMA xbar transposes                    [768, 4096]
    K,V  = row-major projections (lhsT = xT blocks)         [rows, 384]
    kv   accumulated in PSUM over 32 row chunks; a ones-column appended to V
         gives ksum for free.
    Q.T  = feat-major projection (lhsT = Wq, rhs = xT), phi fused into eviction
    z    = 1/(ksum_blockdiag.T @ Q.T), num.T = kvblockdiag.T @ Q.T
    out  = (num.T * z).T @ Wo via lhsT = normed.T blocks -> row-major -> DMA

phi(t) = elu(t)+1 = max(min(exp(t), 1), t+1).
"""

from contextlib import ExitStack

import ml_dtypes
import numpy as np

import concourse.bass as bass
import concourse.mybir as mybir
import concourse.tile as tile
from concourse import bacc
from concourse.bass_utils import run_bass_kernel_spmd

FP = mybir.dt.float32
BF = mybir.dt.bfloat16
ALU = mybir.AluOpType
ACTF = mybir.ActivationFunctionType

P = 128
R = 4096          # sequence rows per core (one full batch element)
F = 768           # input features
H = 6             # heads per core
D = 64            # head dim
G = H * D         # 384 output features per core
KO = F // P       # 6 input-feature chunks
MO = G // P       # 3 output-feature chunks
NCH = R // P      # 32 row chunks of 128
RC = 512          # stage-C row chunk
NRC = R // RC     # 8

N_CORES = 8


def build_nc():
    nc = bacc.Bacc("TRN2", target_bir_lowering=False, debug=False)

    x = nc.dram_tensor("x", [R, F], BF, kind="ExternalInput").ap()
    wq = nc.dram_tensor("wq", [F, G], BF, kind="ExternalInput").ap()
    wk = nc.dram_tensor("wk", [F, G], BF, kind="ExternalInput").ap()
    wv = nc.dram_tensor("wv", [F, G], BF, kind="ExternalInput").ap()
    wo = nc.dram_tensor("wo", [G, F], BF, kind="ExternalInput").ap()
    bq = nc.dram_tensor("bq", [G], FP, kind="ExternalInput").ap()
    bk = nc.dram_tensor("bk", [G], FP, kind="ExternalInput").ap()
    bv = nc.dram_tensor("bv", [G], FP, kind="ExternalInput").ap()
    bo = nc.dram_tensor("bo", [F], FP, kind="ExternalInput").ap()
    # esel[h, p, m] = 1 if h == 2p + (m >= 64): expands z [H, rc] to [128, rc]
    # per head-pair via a tiny matmul (partition-dim broadcast isn't allowed on
    # compute engines).
    esel = nc.dram_tensor("esel", [H, MO, P], BF, kind="ExternalInput").ap()
    y = nc.dram_tensor("y", [R, F], FP, kind="ExternalOutput").ap()

    def bcast_rows(vec_ap, n):
        # DRAM [n] -> partition-broadcast AP [P, n] (partition stride 0)
        return bass.AP(tensor=vec_ap.tensor, offset=vec_ap.offset, ap=[[0, P], [1, n]])

    with tile.TileContext(nc) as tc, ExitStack() as ctx:
        singles = ctx.enter_context(tc.tile_pool(name="singles", bufs=1))
        wpool = ctx.enter_context(tc.tile_pool(name="wpool", bufs=2))

        # biases (fp32, applied on DVE/ACT during psum eviction)
        bk_rep = singles.tile([P, G], FP, tag="bk_rep")
        nc.sync.dma_start(bk_rep, bcast_rows(bk, G))
        bv_rep = singles.tile([P, G], FP, tag="bv_rep")
        nc.sync.dma_start(bv_rep, bcast_rows(bv, G))
        bo_rep = singles.tile([P, F], FP, tag="bo_rep")
        nc.sync.dma_start(bo_rep, bcast_rows(bo, F))
        bq_col = singles.tile([P, MO], FP, tag="bq_col")
        nc.sync.dma_start(bq_col, bq.rearrange("(mo p) -> p mo", p=P))
        bq1_col = singles.tile([P, MO], FP, tag="bq1_col")
        nc.vector.tensor_scalar(bq1_col, bq_col, 1.0, None, op0=ALU.add)
        esel_sb = singles.tile([H, MO, P], BF, tag="esel_sb")
        nc.sync.dma_start(esel_sb, esel)

        # x.T via DMA xbar transpose, resident through stage C
        xt_pool = ctx.enter_context(tc.tile_pool(name="xt", bufs=1))
        xt = xt_pool.tile([P, KO, R], BF)
        for ko in range(KO):
            nc.sync.dma_start_transpose(xt[:, ko, :], x[:, ko * P : (ko + 1) * P])

        # weights for stage A
        wk_sb = wpool.tile([P, KO, G], BF, tag="w")
        nc.sync.dma_start(wk_sb, wk.rearrange("(ko p) g -> p ko g", p=P))
        wv_sb = wpool.tile([P, KO, G], BF, tag="w")
        nc.sync.dma_start(wv_sb, wv.rearrange("(ko p) g -> p ko g", p=P))

        # stage-A outputs that persist into stage C
        kvblk = [
            singles.tile([P, P], BF, tag=f"kvblk{p}", name=f"kvblk{p}")
            for p in range(MO)
        ]
        ksum_mat = singles.tile([P, MO, H], BF, tag="ksum_mat")

        # ---------------- stage A: K, V, kv, ksum ---------------------------
        with ExitStack() as sctx:
            kp_pool = sctx.enter_context(tc.tile_pool(name="kp", bufs=2, space="PSUM"))
            vp_pool = sctx.enter_context(tc.tile_pool(name="vp", bufs=2, space="PSUM"))
            kv_pool = sctx.enter_context(tc.tile_pool(name="kvp", bufs=1, space="PSUM"))
            ksb_pool = sctx.enter_context(tc.tile_pool(name="ksb", bufs=2))
            vsb_pool = sctx.enter_context(tc.tile_pool(name="vsb", bufs=2))
            tmp_pool = sctx.enter_context(tc.tile_pool(name="katmp", bufs=2))

            kv_ps = [
                kv_pool.tile([P, H * (D + 1)], FP, tag=f"kv{p}", name=f"kv{p}")
                for p in range(MO)
            ]

            for i in range(NCH):
                # K projection: rows on partitions
                kps = kp_pool.tile([P, G], FP)
                for ko in range(KO):
                    nc.tensor.matmul(
                        kps,
                        lhsT=xt[:, ko, i * P : (i + 1) * P],
                        rhs=wk_sb[:, ko, :],
                        start=(ko == 0),
                        stop=(ko == KO - 1),
                    )
                # phi(k + bk) = max(min(exp(t), 1), t + 1)
                t1 = tmp_pool.tile([P, G], FP, tag="kt1")
                nc.vector.tensor_tensor(t1, kps, bk_rep, op=ALU.add)
                e = tmp_pool.tile([P, G], FP, tag="ke")
                nc.scalar.activation(e, t1, ACTF.Exp)
                nc.vector.tensor_scalar(e, e, 1.0, None, op0=ALU.min)
                ksb = ksb_pool.tile([P, G], BF)
                nc.vector.tensor_scalar(ksb, t1, 1.0, None, op0=ALU.add)
                nc.vector.tensor_tensor(ksb, ksb, e, op=ALU.max)

                # V projection + bias, with ones column per head for ksum
                vps = vp_pool.tile([P, G], FP)
                for ko in range(KO):
                    nc.tensor.matmul(
                        vps,
                        lhsT=xt[:, ko, i * P : (i + 1) * P],
                        rhs=wv_sb[:, ko, :],
                        start=(ko == 0),
                        stop=(ko == KO - 1),
                    )
                vext = vsb_pool.tile([P, H, D + 1], BF)
                nc.vector.memset(vext[:, :, D : D + 1], 1.0)
                nc.vector.tensor_tensor(
                    vext[:, :, 0:D],
                    vps.rearrange("p (h d) -> p h d", d=D),
                    bv_rep.rearrange("p (h d) -> p h d", d=D),
                    op=ALU.add,
                )

                # kv accumulation: one matmul per head-pair, full V as rhs
                vflat = vext.rearrange("p h e -> p (h e)")
                for p in range(MO):
                    nc.tensor.matmul(
                        kv_ps[p],
                        lhsT=ksb[:, p * P : (p + 1) * P],
                        rhs=vflat,
                        start=(i == 0),
                        stop=(i == NCH - 1),
                    )

            # extract kv block-diagonals and ksum columns
            for p in range(MO):
                nc.vector.memset(kvblk[p], 0.0)
                h0, h1 = 2 * p, 2 * p + 1
                nc.vector.tensor_copy(
                    kvblk[p][0:D, 0:D], kv_ps[p][0:D, h0 * (D + 1) : h0 * (D + 1) + D]
                )
                nc.vector.tensor_copy(
                    kvblk[p][D:P, D:P], kv_ps[p][D:P, h1 * (D + 1) : h1 * (D + 1) + D]
                )
            nc.vector.memset(ksum_mat, 0.0)
            for h in range(H):
                p = h // 2
                r0 = (h % 2) * D
                nc.vector.tensor_copy(
                    ksum_mat[r0 : r0 + D, p, h : h + 1], kv_ps[p][r0 : r0 + D, D : D + 1]
                )

        # weights for stage C
        wq_sb = wpool.tile([P, KO, G], BF, tag="w")
        nc.sync.dma_start(wq_sb, wq.rearrange("(ko p) g -> p ko g", p=P))
        wo_sb = wpool.tile([P, MO, F], BF, tag="w")
        nc.sync.dma_start(wo_sb, wo.rearrange("(mo p) f -> p mo f", p=P))

        # ---------------- stage C: Q.T, z, num, output ----------------------
        with ExitStack() as sctx:
            qp_pool = sctx.enter_context(tc.tile_pool(name="qp", bufs=2, space="PSUM"))
            zp_pool = sctx.enter_context(tc.tile_pool(name="zp", bufs=1, space="PSUM"))
            np_pool = sctx.enter_context(tc.tile_pool(name="nump", bufs=2, space="PSUM"))
            zx_pool = sctx.enter_context(tc.tile_pool(name="zx", bufs=1, space="PSUM"))
            op_pool = sctx.enter_context(tc.tile_pool(name="outp", bufs=1, space="PSUM"))
            qt_pool = sctx.enter_context(tc.tile_pool(name="qt", bufs=2))
            qe_pool = sctx.enter_context(tc.tile_pool(name="qe", bufs=3))
            zr_pool = sctx.enter_context(tc.tile_pool(name="zr", bufs=2))
            zxs_pool = sctx.enter_context(tc.tile_pool(name="zxs", bufs=2))
            nrm_pool = sctx.enter_context(tc.tile_pool(name="nrm", bufs=2))
            out_pool = sctx.enter_context(tc.tile_pool(name="osb", bufs=3))

            for rc in range(NRC):
                rs = slice(rc * RC, (rc + 1) * RC)
                # Q.T chunk [128, MO, RC], phi fused into eviction
                qt_rc = qt_pool.tile([P, MO, RC], BF)
                for mo in range(MO):
                    qps = qp_pool.tile([P, RC], FP)
                    for ko in range(KO):
                        nc.tensor.matmul(
                            qps,
                            lhsT=wq_sb[:, ko, mo * P : (mo + 1) * P],
                            rhs=xt[:, ko, rs],
                            start=(ko == 0),
                            stop=(ko == KO - 1),
                        )
                    e = qe_pool.tile([P, RC], FP)
                    nc.scalar.activation(e, qps, ACTF.Exp, bias=bq_col[:, mo : mo + 1])
                    nc.vector.tensor_scalar(e, e, 1.0, None, op0=ALU.min)
                    qslice = qt_rc[:, mo, :]
                    nc.vector.tensor_scalar(
                        qslice, qps, bq1_col[:, mo : mo + 1], None, op0=ALU.add
                    )
                    nc.vector.tensor_tensor(qslice, qslice, e, op=ALU.max)

                # z = 1 / (q . ksum)
                zps = zp_pool.tile([H, RC], FP)
                for mo in range(MO):
                    nc.tensor.matmul(
                        zps,
                        lhsT=ksum_mat[:, mo, :],
                        rhs=qt_rc[:, mo, :],
                        start=(mo == 0),
                        stop=(mo == MO - 1),
                    )
                zr = zr_pool.tile([H, RC], BF)
                with nc.allow_low_precision(reason="bf16 rounding of 1/z"):
                    nc.vector.reciprocal(zr, zps)

                # num.T per head pair; z expanded across partitions via esel
                nrm = nrm_pool.tile([P, MO, RC], BF)
                for p in range(MO):
                    nps = np_pool.tile([P, RC], FP)
                    nc.tensor.matmul(nps, lhsT=kvblk[p], rhs=qt_rc[:, p, :])
                    zxp = zx_pool.tile([P, RC], FP)
                    nc.tensor.matmul(zxp, lhsT=esel_sb[:, p, :], rhs=zr)
                    zxs = zxs_pool.tile([P, RC], FP)
                    nc.scalar.copy(zxs, zxp)
                    nc.vector.tensor_tensor(nrm[:, p, :], nps, zxs, op=ALU.mult)

                # output projection, row-major
                for sub in range(4):
                    o_ps = [
                        op_pool.tile([P, F // 2], FP, tag=f"op{hh}", name=f"ops{hh}")
                        for hh in range(2)
                    ]
                    for hh in range(2):
                        for p in range(MO):
                            nc.tensor.matmul(
                                o_ps[hh],
                                lhsT=nrm[:, p, sub * P : (sub + 1) * P],
                                rhs=wo_sb[:, p, hh * (F // 2) : (hh + 1) * (F // 2)],
                                start=(p == 0),
                                stop=(p == MO - 1),
                            )
                    osb = out_pool.tile([P, F], FP)
                    nc.vector.tensor_tensor(
                        osb[:, 0 : F // 2], o_ps[0], bo_rep[:, 0 : F // 2], op=ALU.add
                    )
                    nc.vector.tensor_tensor(
                        osb[:, F // 2 : F], o_ps[1], bo_rep[:, F // 2 : F], op=ALU.add
                    )
                    r0 = rc * RC + sub * P
                    nc.sync.dma_start(y[r0 : r0 + P, :], osb)

    nc.compile()
    return nc


def make_in_maps(x, Wq, bq, Wk, bk, Wv, bv, Wo, bo):
    """Shard full inputs into the 8 per-core input maps."""
    f32 = lambda a: np.ascontiguousarray(np.asarray(a, dtype=np.float32))
    bf16 = lambda a: np.ascontiguousarray(np.asarray(a).astype(ml_dtypes.bfloat16))
    x, Wq, Wk, Wv, Wo = map(bf16, (x, Wq, Wk, Wv, Wo))
    bq, bk, bv, bo = map(f32, (bq, bk, bv, bo))
    zeros_f = np.zeros_like(bo)
    esel = np.zeros((H, MO, P), dtype=ml_dtypes.bfloat16)
    for h in range(H):
        esel[h, h // 2, (h % 2) * D : (h % 2 + 1) * D] = 1.0
    in_maps = []
    for c in range(N_CORES):
        b, g = divmod(c, 2)
        sl = slice(g * G, (g + 1) * G)
        in_maps.append(
            {
                "x": x[b],
                "wq": bf16(Wq[:, sl]),
                "wk": bf16(Wk[:, sl]),
                "wv": bf16(Wv[:, sl]),
                "wo": bf16(Wo[sl, :]),
                "bq": f32(bq[sl]),
                "bk": f32(bk[sl]),
                "bv": f32(bv[sl]),
                "bo": bo if g == 0 else zeros_f,
                "esel": esel,
            }
        )
    return in_maps


def unshard(core_outs):
    """Sum the two row-parallel partials per batch element."""
    return np.stack(
        [core_outs[2 * b] + core_outs[2 * b + 1] for b in range(N_CORES // 2)]
    )


_NC_CACHE = {}


def get_nc():
    if "nc" not in _NC_CACHE:
        _NC_CACHE["nc"] = build_nc()
    return _NC_CACHE["nc"]


def run(inputs, trace=False, **kwargs):
    nc = get_nc()
    in_maps = make_in_maps(**inputs)
    res = run_bass_kernel_spmd(
        nc, in_maps, core_ids=list(range(N_CORES)), trace=trace, **kwargs
    )
    out = unshard([r["y"] for r in res.results])
    return out, res


def kernel(**inputs):
    out, _ = run(inputs, trace=False)
    return out
